# revision 3
# baseline (speedup 1.0000x reference)
"""Trainium2 Bass kernel for im2col conv2d + bias + channel-pack.

Semantics (matches the reference):
    out[c, w] = sum_k enc_x[w, k] * weight[c, k] + bias[c],  flattened to [C*W].

Strategy:
  - Shard the window dimension W=1048576 across 8 cores (131072 windows each).
  - Host-side: transpose enc_x to [K, W] (so the contraction dim K=49 lands on
    SBUF partitions) and cast to fp16 (halves HBM traffic; PE accumulates fp32).
  - Device-side: stationary operand is a block-diagonal [2K, 2C] weight matrix,
    so each matmul computes TWO 512-window chunks at once and the output tile
    occupies 64 partitions (keeps the scalar-engine bias/copy off the critical
    path). Bias is fused into the PSUM->SBUF copy via activation(Identity, bias).
  - Memory-bound regime: per-core HBM traffic = 12.8 MB in + 16.8 MB out.
"""

import os

import numpy as np

K = 49
C = 32
WINDOWS_NB = 1048576
N_CORES = 8
W_CORE = WINDOWS_NB // N_CORES  # 131072

# Device tiling parameters (full-size problem).
F = 16384  # windows per half input tile  (x_tile is [2K, F], covers 2F windows)
OSB = 4096  # output sbuf tile free dim
GROUP = 2048  # psum tile free dim (4 matmuls of 512)
NMM = 512  # matmul moving free dim (one PSUM bank of fp32)

_PROGRAM_CACHE: dict = {}
LAST_RESULT = None  # BassKernelResults of the most recent run (for test harness)


def build_program(w_core=W_CORE, f=F, osb=OSB, group=GROUP, nmm=NMM):
    import concourse.tile as tile
    from concourse import bacc, mybir

    assert w_core % (2 * f) == 0 and f % osb == 0 and osb % group == 0
    assert group % nmm == 0
    n_outer = w_core // (2 * f)

    nc = bacc.Bacc("TRN2", debug=False, num_devices=N_CORES)
    xt = nc.dram_tensor("xt", [K, w_core], mybir.dt.float16, kind="ExternalInput")
    w2 = nc.dram_tensor("w2", [2 * K, 2 * C], mybir.dt.float16, kind="ExternalInput")
    br = nc.dram_tensor("br", [2 * C, 1], mybir.dt.float32, kind="ExternalInput")
    out = nc.dram_tensor("out", [C, w_core], mybir.dt.float32, kind="ExternalOutput")

    with tile.TileContext(nc) as tc:
        with tc.tile_pool(name="const", bufs=1) as cpool, \
             tc.tile_pool(name="xin", bufs=3) as xpool, \
             tc.tile_pool(name="osb", bufs=3) as opool, \
             tc.tile_pool(name="ps", bufs=2, space="PSUM") as ppool:
            w_sb = cpool.tile([2 * K, 2 * C], mybir.dt.float16)
            nc.sync.dma_start(out=w_sb, in_=w2.ap())
            b_sb = cpool.tile([2 * C, 1], mybir.dt.float32)
            nc.sync.dma_start(out=b_sb, in_=br.ap())

            xt_ap = xt.ap()
            # out element [c, i*(2f) + j*f + s]  <->  AP dims [i, j, c, s]
            out_r = out.ap().rearrange("c (i j s) -> i j c s", i=n_outer, j=2)

            for it in range(n_outer):
                a0 = it * 2 * f
                x_tile = xpool.tile([2 * K, f], mybir.dt.float16)
                nc.sync.dma_start(out=x_tile[0:K, :], in_=xt_ap[:, a0:a0 + f])
                nc.sync.dma_start(out=x_tile[K:2 * K, :], in_=xt_ap[:, a0 + f:a0 + 2 * f])
                for ob in range(f // osb):
                    o_tile = opool.tile([2 * C, osb], mybir.dt.float32)
                    for g in range(osb // group):
                        ps = ppool.tile([2 * C, group], mybir.dt.float32)
                        for n in range(group // nmm):
                            col = ob * osb + g * group + n * nmm
                            nc.tensor.matmul(
                                ps[:, n * nmm:(n + 1) * nmm],
                                w_sb,
                                x_tile[:, col:col + nmm],
                                start=True,
                                stop=True,
                            )
                        nc.scalar.activation(
                            o_tile[:, g * group:(g + 1) * group],
                            ps,
                            mybir.ActivationFunctionType.Identity,
                            bias=b_sb,
                            scale=1.0,
                        )
                    nc.sync.dma_start(
                        out=out_r[it, :, :, ob * osb:(ob + 1) * osb],
                        in_=o_tile,
                    )
    nc.compile()
    return nc


def _get_program():
    key = (W_CORE, F, OSB, GROUP, NMM)
    if key not in _PROGRAM_CACHE:
        _PROGRAM_CACHE[key] = build_program()
    return _PROGRAM_CACHE[key]


def prepare_inputs(enc_x, weight, bias):
    """Host-side prep: per-core transposed fp16 shards + block-diag weights."""
    enc_x = np.asarray(enc_x, dtype=np.float32)
    weight = np.asarray(weight, dtype=np.float32)
    bias = np.asarray(bias, dtype=np.float32)

    wflat = weight.reshape(C, K)
    w2 = np.zeros((2 * K, 2 * C), dtype=np.float16)
    w2[0:K, 0:C] = wflat.T.astype(np.float16)
    w2[K:2 * K, C:2 * C] = wflat.T.astype(np.float16)
    br = np.tile(bias, 2)[:, None].astype(np.float32)

    x16 = enc_x.astype(np.float16)
    shards = [
        np.ascontiguousarray(x16[i * W_CORE:(i + 1) * W_CORE].T)
        for i in range(N_CORES)
    ]
    return shards, w2, br


def kernel(enc_x, weight, bias, windows_nb=None):
    global LAST_RESULT
    from concourse import bass_utils

    shards, w2, br = prepare_inputs(enc_x, weight, bias)
    nc = _get_program()
    in_maps = [{"xt": shards[i], "w2": w2, "br": br} for i in range(N_CORES)]
    trace = bool(int(os.environ.get("BASS_KERNEL_TRACE", "0")))
    tmpdir = os.environ.get("BASS_KERNEL_TMPDIR") or None
    res = bass_utils.run_bass_kernel_spmd(
        nc, in_maps, core_ids=list(range(N_CORES)), trace=trace, tmpdir=tmpdir
    )
    LAST_RESULT = res
    outs = [res.results[i]["out"] for i in range(N_CORES)]
    return np.concatenate(outs, axis=1).reshape(-1)


# revision 4
# speedup vs baseline: 2.5545x; 2.5545x over previous
"""Trainium2 Bass kernel for im2col conv2d + bias + channel-pack.

Semantics (matches the reference):
    out[c, w] = sum_k enc_x[w, k] * weight[c, k] + bias[c],  flattened to [C*W].

Strategy:
  - Shard the window dimension W=1048576 across 8 cores (131072 windows each).
  - Host-side: transpose enc_x to [K, W] (so the contraction dim K=49 lands on
    SBUF partitions) and cast to fp16 (halves HBM traffic; PE accumulates fp32).
  - Device-side: stationary operand is a block-diagonal [2K, 2C] weight matrix,
    so each matmul computes TWO 512-window chunks at once and the output tile
    occupies 64 partitions (keeps the scalar-engine bias/copy off the critical
    path). Bias is fused into the PSUM->SBUF copy via activation(Identity, bias).
  - Memory-bound regime: per-core HBM traffic = 12.8 MB in + 16.8 MB out.
"""

import os

import numpy as np

K = 49
C = 32
WINDOWS_NB = 1048576
N_CORES = 8
W_CORE = WINDOWS_NB // N_CORES  # 131072

# Device tiling parameters (full-size problem).
F = 16384  # windows per half input tile  (x_tile is [2K, F], covers 2F windows)
OSB = 4096  # output sbuf tile free dim
GROUP = 2048  # psum tile free dim (4 matmuls of 512)
NMM = 512  # matmul moving free dim (one PSUM bank of fp32)

_PROGRAM_CACHE: dict = {}
LAST_RESULT = None  # BassKernelResults of the most recent run (for test harness)


def build_program(w_core=W_CORE, f=F, osb=OSB, group=GROUP, nmm=NMM):
    import concourse.tile as tile
    from concourse import bacc, mybir

    assert w_core % (2 * f) == 0 and f % osb == 0 and osb % group == 0
    assert group % nmm == 0
    n_outer = w_core // (2 * f)

    nc = bacc.Bacc("TRN2", debug=False, num_devices=N_CORES)
    xt = nc.dram_tensor("xt", [K, w_core], mybir.dt.float16, kind="ExternalInput")
    w2 = nc.dram_tensor("w2", [2 * K, 2 * C], mybir.dt.float16, kind="ExternalInput")
    br = nc.dram_tensor("br", [2 * C, 1], mybir.dt.float32, kind="ExternalInput")
    out = nc.dram_tensor("out", [C, w_core], mybir.dt.float32, kind="ExternalOutput")

    with tile.TileContext(nc) as tc:
        with tc.tile_pool(name="const", bufs=1) as cpool, \
             tc.tile_pool(name="xin", bufs=3) as xpool, \
             tc.tile_pool(name="osb", bufs=3) as opool, \
             tc.tile_pool(name="ps", bufs=2, space="PSUM") as ppool:
            w_sb = cpool.tile([2 * K, 2 * C], mybir.dt.float16)
            nc.sync.dma_start(out=w_sb, in_=w2.ap())
            b_sb = cpool.tile([2 * C, 1], mybir.dt.float32)
            nc.sync.dma_start(out=b_sb, in_=br.ap())

            xt_ap = xt.ap()
            # out element [c, i*(2f) + j*f + s]  <->  AP dims [i, j, c, s]
            out_r = out.ap().rearrange("c (i j s) -> i j c s", i=n_outer, j=2)

            for it in range(n_outer):
                a0 = it * 2 * f
                x_tile = xpool.tile([2 * K, f], mybir.dt.float16)
                nc.sync.dma_start(out=x_tile[0:K, :], in_=xt_ap[:, a0:a0 + f])
                nc.sync.dma_start(out=x_tile[K:2 * K, :], in_=xt_ap[:, a0 + f:a0 + 2 * f])
                for ob in range(f // osb):
                    o_tile = opool.tile([2 * C, osb], mybir.dt.float32)
                    for g in range(osb // group):
                        ps = ppool.tile([2 * C, group], mybir.dt.float32)
                        for n in range(group // nmm):
                            col = ob * osb + g * group + n * nmm
                            nc.tensor.matmul(
                                ps[:, n * nmm:(n + 1) * nmm],
                                w_sb,
                                x_tile[:, col:col + nmm],
                                start=True,
                                stop=True,
                            )
                        nc.scalar.activation(
                            o_tile[:, g * group:(g + 1) * group],
                            ps,
                            mybir.ActivationFunctionType.Identity,
                            bias=b_sb,
                            scale=1.0,
                        )
                    # One DMA per j-half: keeps the DRAM-side outer dim at 32
                    # (c), so HWDGE spreads descriptors across all engines
                    # instead of piling everything on two.
                    for j in range(2):
                        nc.sync.dma_start(
                            out=out_r[it, j, :, ob * osb:(ob + 1) * osb],
                            in_=o_tile[j * C:(j + 1) * C, :],
                        )
    nc.compile()
    return nc


def _get_program():
    key = (W_CORE, F, OSB, GROUP, NMM)
    if key not in _PROGRAM_CACHE:
        _PROGRAM_CACHE[key] = build_program()
    return _PROGRAM_CACHE[key]


def prepare_inputs(enc_x, weight, bias):
    """Host-side prep: per-core transposed fp16 shards + block-diag weights."""
    enc_x = np.asarray(enc_x, dtype=np.float32)
    weight = np.asarray(weight, dtype=np.float32)
    bias = np.asarray(bias, dtype=np.float32)

    wflat = weight.reshape(C, K)
    w2 = np.zeros((2 * K, 2 * C), dtype=np.float16)
    w2[0:K, 0:C] = wflat.T.astype(np.float16)
    w2[K:2 * K, C:2 * C] = wflat.T.astype(np.float16)
    br = np.tile(bias, 2)[:, None].astype(np.float32)

    x16 = enc_x.astype(np.float16)
    shards = [
        np.ascontiguousarray(x16[i * W_CORE:(i + 1) * W_CORE].T)
        for i in range(N_CORES)
    ]
    return shards, w2, br


def kernel(enc_x, weight, bias, windows_nb=None):
    global LAST_RESULT
    from concourse import bass_utils

    shards, w2, br = prepare_inputs(enc_x, weight, bias)
    nc = _get_program()
    in_maps = [{"xt": shards[i], "w2": w2, "br": br} for i in range(N_CORES)]
    trace = bool(int(os.environ.get("BASS_KERNEL_TRACE", "0")))
    tmpdir = os.environ.get("BASS_KERNEL_TMPDIR") or None
    res = bass_utils.run_bass_kernel_spmd(
        nc, in_maps, core_ids=list(range(N_CORES)), trace=trace, tmpdir=tmpdir
    )
    LAST_RESULT = res
    outs = [res.results[i]["out"] for i in range(N_CORES)]
    return np.concatenate(outs, axis=1).reshape(-1)


# revision 9
# speedup vs baseline: 2.8149x; 1.1020x over previous
"""Trainium2 Bass kernel for im2col conv2d + bias + channel-pack.

Semantics (matches the reference):
    out[c, w] = sum_k enc_x[w, k] * weight[c, k] + bias[c],  flattened to [C*W].

Strategy:
  - Shard the window dimension W=1048576 across 8 cores (131072 windows each).
  - Host-side: transpose enc_x to [K, W] (so the contraction dim K=49 lands on
    SBUF partitions) and cast to fp16 (halves HBM traffic; PE accumulates fp32).
  - Device-side: stationary operand is a block-diagonal [2K, 2C] weight matrix,
    so each matmul computes TWO 512-window chunks at once and the output tile
    occupies 64 partitions (keeps the scalar-engine bias/copy off the critical
    path). Bias is fused into the PSUM->SBUF copy via activation(Identity, bias).
  - Memory-bound regime: per-core HBM traffic = 12.8 MB in + 16.8 MB out.
"""

import os

import numpy as np

K = 49
C = 32
WINDOWS_NB = 1048576
N_CORES = 8
W_CORE = WINDOWS_NB // N_CORES  # 131072

# Device tiling parameters (full-size problem).
F = 16384  # windows per half input tile  (x_tile is [2K, F], covers 2F windows)
OSB = 8192  # output sbuf tile free dim
GROUP = 2048  # psum tile free dim (4 matmuls of 512)
NMM = 512  # matmul moving free dim (one PSUM bank of fp32)

_PROGRAM_CACHE: dict = {}
LAST_RESULT = None  # BassKernelResults of the most recent run (for test harness)


def build_program(w_core=W_CORE, f=F, osb=OSB, group=GROUP, nmm=NMM):
    import concourse.tile as tile
    from concourse import bacc, mybir

    assert w_core % (2 * f) == 0 and f % osb == 0 and osb % group == 0
    assert group % nmm == 0
    n_outer = w_core // (2 * f)

    nc = bacc.Bacc("TRN2", debug=False, num_devices=N_CORES)
    xt = nc.dram_tensor("xt", [K, w_core], mybir.dt.float16, kind="ExternalInput")
    w2 = nc.dram_tensor("w2", [2 * K, 2 * C], mybir.dt.float16, kind="ExternalInput")
    br = nc.dram_tensor("br", [2 * C, 1], mybir.dt.float32, kind="ExternalInput")
    # fp16 output (upcast on host): halves HBM write traffic, which is the
    # dominant cost in this memory-bound kernel.
    out = nc.dram_tensor("out", [C, w_core], mybir.dt.float16, kind="ExternalOutput")

    with tile.TileContext(nc) as tc:
        with tc.tile_pool(name="const", bufs=1) as cpool, \
             tc.tile_pool(name="xin", bufs=3) as xpool, \
             tc.tile_pool(name="osb", bufs=3) as opool, \
             tc.tile_pool(name="ps", bufs=2, space="PSUM") as ppool:
            w_sb = cpool.tile([2 * K, 2 * C], mybir.dt.float16)
            nc.sync.dma_start(out=w_sb, in_=w2.ap())
            b_sb = cpool.tile([2 * C, 1], mybir.dt.float32)
            nc.sync.dma_start(out=b_sb, in_=br.ap())

            xt_ap = xt.ap()
            # out element [c, i*(2f) + j*f + s]  <->  AP dims [i, j, c, s]
            out_r = out.ap().rearrange("c (i j s) -> i j c s", i=n_outer, j=2)

            for it in range(n_outer):
                a0 = it * 2 * f
                x_tile = xpool.tile([2 * K, f], mybir.dt.float16)
                # Split each half-load into 48+1 rows: HWDGE fans a DMA's
                # descriptors over the largest engine count dividing the outer
                # dim (49 -> only 7 engines; 48 -> all 16).
                nc.sync.dma_start(out=x_tile[0:48, :], in_=xt_ap[0:48, a0:a0 + f])
                nc.sync.dma_start(out=x_tile[48:K, :], in_=xt_ap[48:K, a0:a0 + f])
                nc.sync.dma_start(out=x_tile[K:K + 48, :], in_=xt_ap[0:48, a0 + f:a0 + 2 * f])
                nc.sync.dma_start(out=x_tile[K + 48:2 * K, :], in_=xt_ap[48:K, a0 + f:a0 + 2 * f])
                for ob in range(f // osb):
                    o_tile = opool.tile([2 * C, osb], mybir.dt.float16)
                    for g in range(osb // group):
                        ps = ppool.tile([2 * C, group], mybir.dt.float32)
                        for n in range(group // nmm):
                            col = ob * osb + g * group + n * nmm
                            nc.tensor.matmul(
                                ps[:, n * nmm:(n + 1) * nmm],
                                w_sb,
                                x_tile[:, col:col + nmm],
                                start=True,
                                stop=True,
                            )
                        nc.scalar.activation(
                            o_tile[:, g * group:(g + 1) * group],
                            ps,
                            mybir.ActivationFunctionType.Identity,
                            bias=b_sb,
                            scale=1.0,
                        )
                    # One DMA per j-half: keeps the DRAM-side outer dim at 32
                    # (c), so HWDGE spreads descriptors across all engines
                    # instead of piling everything on two.
                    for j in range(2):
                        nc.sync.dma_start(
                            out=out_r[it, j, :, ob * osb:(ob + 1) * osb],
                            in_=o_tile[j * C:(j + 1) * C, :],
                        )
    nc.compile()
    return nc


def _get_program():
    key = (W_CORE, F, OSB, GROUP, NMM)
    if key not in _PROGRAM_CACHE:
        _PROGRAM_CACHE[key] = build_program()
    return _PROGRAM_CACHE[key]


def prepare_inputs(enc_x, weight, bias):
    """Host-side prep: per-core transposed fp16 shards + block-diag weights."""
    enc_x = np.asarray(enc_x, dtype=np.float32)
    weight = np.asarray(weight, dtype=np.float32)
    bias = np.asarray(bias, dtype=np.float32)

    wflat = weight.reshape(C, K)
    w2 = np.zeros((2 * K, 2 * C), dtype=np.float16)
    w2[0:K, 0:C] = wflat.T.astype(np.float16)
    w2[K:2 * K, C:2 * C] = wflat.T.astype(np.float16)
    br = np.tile(bias, 2)[:, None].astype(np.float32)

    x16 = enc_x.astype(np.float16)
    shards = [
        np.ascontiguousarray(x16[i * W_CORE:(i + 1) * W_CORE].T)
        for i in range(N_CORES)
    ]
    return shards, w2, br


def kernel(enc_x, weight, bias, windows_nb=None):
    global LAST_RESULT
    from concourse import bass_utils

    shards, w2, br = prepare_inputs(enc_x, weight, bias)
    nc = _get_program()
    in_maps = [{"xt": shards[i], "w2": w2, "br": br} for i in range(N_CORES)]
    trace = bool(int(os.environ.get("BASS_KERNEL_TRACE", "0")))
    tmpdir = os.environ.get("BASS_KERNEL_TMPDIR") or None
    res = bass_utils.run_bass_kernel_spmd(
        nc, in_maps, core_ids=list(range(N_CORES)), trace=trace, tmpdir=tmpdir
    )
    LAST_RESULT = res
    outs = [res.results[i]["out"] for i in range(N_CORES)]
    return np.concatenate(outs, axis=1).astype(np.float32).reshape(-1)


# revision 10
# speedup vs baseline: 3.6424x; 1.2940x over previous
"""Trainium2 Bass kernel for im2col conv2d + bias + channel-pack.

Semantics (matches the reference):
    out[c, w] = sum_k enc_x[w, k] * weight[c, k] + bias[c],  flattened to [C*W].

Strategy:
  - Shard the window dimension W=1048576 across 8 cores (131072 windows each).
  - Host-side: transpose enc_x to [K, W] (so the contraction dim K=49 lands on
    SBUF partitions) and cast to fp16 (halves HBM traffic; PE accumulates fp32).
  - Device-side: stationary operand is a block-diagonal [2K, 2C] weight matrix,
    so each matmul computes TWO 512-window chunks at once and the output tile
    occupies 64 partitions (keeps the scalar-engine bias/copy off the critical
    path). Bias is fused into the PSUM->SBUF copy via activation(Identity, bias).
  - Memory-bound regime: per-core HBM traffic = 12.8 MB in + 16.8 MB out.
"""

import os

import numpy as np

K = 49
C = 32
WINDOWS_NB = 1048576
N_CORES = 8
W_CORE = WINDOWS_NB // N_CORES  # 131072

# Device tiling parameters (full-size problem).
F = 16384  # windows per half input tile  (x_tile is [2K, F], covers 2F windows)
OSB = 8192  # output sbuf tile free dim
GROUP = 2048  # psum tile free dim (4 matmuls of 512)
NMM = 512  # matmul moving free dim (one PSUM bank of fp32)

_PROGRAM_CACHE: dict = {}
LAST_RESULT = None  # BassKernelResults of the most recent run (for test harness)


def build_program(w_core=W_CORE, f=F, osb=OSB, group=GROUP, nmm=NMM):
    import concourse.tile as tile
    from concourse import bacc, mybir

    assert w_core % (2 * f) == 0 and f % osb == 0 and osb % group == 0
    assert group % nmm == 0
    n_outer = w_core // (2 * f)

    nc = bacc.Bacc("TRN2", debug=False, num_devices=N_CORES)
    xt = nc.dram_tensor("xt", [K, w_core], mybir.dt.float16, kind="ExternalInput")
    w2 = nc.dram_tensor("w2", [2 * K, 2 * C], mybir.dt.float16, kind="ExternalInput")
    br = nc.dram_tensor("br", [2 * C, 1], mybir.dt.float32, kind="ExternalInput")
    # fp16 output (upcast on host): halves HBM write traffic, which is the
    # dominant cost in this memory-bound kernel.
    out = nc.dram_tensor("out", [C, w_core], mybir.dt.float16, kind="ExternalOutput")

    with tile.TileContext(nc) as tc:
        with tc.tile_pool(name="const", bufs=1) as cpool, \
             tc.tile_pool(name="xin", bufs=3) as xpool, \
             tc.tile_pool(name="osb", bufs=3) as opool, \
             tc.tile_pool(name="ps", bufs=2, space="PSUM") as ppool:
            w_sb = cpool.tile([2 * K, 2 * C], mybir.dt.float16)
            nc.sync.dma_start(out=w_sb, in_=w2.ap())
            b_sb = cpool.tile([2 * C, 1], mybir.dt.float32)
            nc.sync.dma_start(out=b_sb, in_=br.ap())

            xt_ap = xt.ap()
            # out element [c, i*(2f) + j*f + s]  <->  AP dims [i, j, c, s]
            out_r = out.ap().rearrange("c (i j s) -> i j c s", i=n_outer, j=2)

            for it in range(n_outer):
                a0 = it * 2 * f
                x_tile = xpool.tile([2 * K, f], mybir.dt.float16)
                # Split each half-load into 48+1 rows: HWDGE fans a DMA's
                # descriptors over the largest engine count dividing the outer
                # dim (49 -> only 7 engines; 48 -> all 16).
                nc.sync.dma_start(out=x_tile[0:48, :], in_=xt_ap[0:48, a0:a0 + f])
                nc.sync.dma_start(out=x_tile[48:K, :], in_=xt_ap[48:K, a0:a0 + f])
                nc.sync.dma_start(out=x_tile[K:K + 48, :], in_=xt_ap[0:48, a0 + f:a0 + 2 * f])
                nc.sync.dma_start(out=x_tile[K + 48:2 * K, :], in_=xt_ap[48:K, a0 + f:a0 + 2 * f])
                for ob in range(f // osb):
                    o_tile = opool.tile([2 * C, osb], mybir.dt.float16)
                    for g in range(osb // group):
                        ps = ppool.tile([2 * C, group], mybir.dt.float32)
                        for n in range(group // nmm):
                            col = ob * osb + g * group + n * nmm
                            nc.tensor.matmul(
                                ps[:, n * nmm:(n + 1) * nmm],
                                w_sb,
                                x_tile[:, col:col + nmm],
                                start=True,
                                stop=True,
                            )
                        nc.scalar.activation(
                            o_tile[:, g * group:(g + 1) * group],
                            ps,
                            mybir.ActivationFunctionType.Identity,
                            bias=b_sb,
                            scale=1.0,
                        )
                    # One DMA per j-half: keeps the DRAM-side outer dim at 32
                    # (c), so HWDGE spreads descriptors across all engines
                    # instead of piling everything on two. Issued on the
                    # scalar-engine HWDGE ring so stores waiting on ACT can't
                    # head-of-line-block the input loads on the sync ring.
                    for j in range(2):
                        nc.scalar.dma_start(
                            out=out_r[it, j, :, ob * osb:(ob + 1) * osb],
                            in_=o_tile[j * C:(j + 1) * C, :],
                        )
    nc.compile()
    return nc


def _get_program():
    key = (W_CORE, F, OSB, GROUP, NMM)
    if key not in _PROGRAM_CACHE:
        _PROGRAM_CACHE[key] = build_program()
    return _PROGRAM_CACHE[key]


def prepare_inputs(enc_x, weight, bias):
    """Host-side prep: per-core transposed fp16 shards + block-diag weights."""
    enc_x = np.asarray(enc_x, dtype=np.float32)
    weight = np.asarray(weight, dtype=np.float32)
    bias = np.asarray(bias, dtype=np.float32)

    wflat = weight.reshape(C, K)
    w2 = np.zeros((2 * K, 2 * C), dtype=np.float16)
    w2[0:K, 0:C] = wflat.T.astype(np.float16)
    w2[K:2 * K, C:2 * C] = wflat.T.astype(np.float16)
    br = np.tile(bias, 2)[:, None].astype(np.float32)

    x16 = enc_x.astype(np.float16)
    shards = [
        np.ascontiguousarray(x16[i * W_CORE:(i + 1) * W_CORE].T)
        for i in range(N_CORES)
    ]
    return shards, w2, br


def kernel(enc_x, weight, bias, windows_nb=None):
    global LAST_RESULT
    from concourse import bass_utils

    shards, w2, br = prepare_inputs(enc_x, weight, bias)
    nc = _get_program()
    in_maps = [{"xt": shards[i], "w2": w2, "br": br} for i in range(N_CORES)]
    trace = bool(int(os.environ.get("BASS_KERNEL_TRACE", "0")))
    tmpdir = os.environ.get("BASS_KERNEL_TMPDIR") or None
    res = bass_utils.run_bass_kernel_spmd(
        nc, in_maps, core_ids=list(range(N_CORES)), trace=trace, tmpdir=tmpdir
    )
    LAST_RESULT = res
    outs = [res.results[i]["out"] for i in range(N_CORES)]
    return np.concatenate(outs, axis=1).astype(np.float32).reshape(-1)


# revision 12
# speedup vs baseline: 3.7353x; 1.0255x over previous
"""Trainium2 Bass kernel for im2col conv2d + bias + channel-pack.

Semantics (matches the reference):
    out[c, w] = sum_k enc_x[w, k] * weight[c, k] + bias[c],  flattened to [C*W].

Strategy:
  - Shard the window dimension W=1048576 across 8 cores (131072 windows each).
  - Host-side: transpose enc_x to [K, W] (so the contraction dim K=49 lands on
    SBUF partitions) and cast to fp16 (halves HBM traffic; PE accumulates fp32).
  - Device-side: stationary operand is a block-diagonal [2K, 2C] weight matrix,
    so each matmul computes TWO 512-window chunks at once and the output tile
    occupies 64 partitions (keeps the scalar-engine bias/copy off the critical
    path). Bias is fused into the PSUM->SBUF copy via activation(Identity, bias).
  - Memory-bound regime: per-core HBM traffic = 12.8 MB in + 16.8 MB out.
"""

import os

import numpy as np

K = 49
C = 32
WINDOWS_NB = 1048576
N_CORES = 8
W_CORE = WINDOWS_NB // N_CORES  # 131072

# Device tiling parameters (full-size problem).
F = 8192  # windows per half input tile  (x_tile is [2K, F], covers 2F windows)
OSB = 8192  # output sbuf tile free dim
GROUP = 2048  # psum tile free dim (4 matmuls of 512)
NMM = 512  # matmul moving free dim (one PSUM bank of fp32)

_PROGRAM_CACHE: dict = {}
LAST_RESULT = None  # BassKernelResults of the most recent run (for test harness)


def build_program(w_core=W_CORE, f=F, osb=OSB, group=GROUP, nmm=NMM):
    import concourse.tile as tile
    from concourse import bacc, mybir

    assert w_core % (2 * f) == 0 and f % osb == 0 and osb % group == 0
    assert group % nmm == 0
    n_outer = w_core // (2 * f)

    nc = bacc.Bacc("TRN2", debug=False, num_devices=N_CORES)
    xt = nc.dram_tensor("xt", [K, w_core], mybir.dt.float16, kind="ExternalInput")
    w2 = nc.dram_tensor("w2", [2 * K, 2 * C], mybir.dt.float16, kind="ExternalInput")
    br = nc.dram_tensor("br", [2 * C, 1], mybir.dt.float32, kind="ExternalInput")
    # fp16 output (upcast on host): halves HBM write traffic, which is the
    # dominant cost in this memory-bound kernel.
    out = nc.dram_tensor("out", [C, w_core], mybir.dt.float16, kind="ExternalOutput")

    with tile.TileContext(nc) as tc:
        with tc.tile_pool(name="const", bufs=1) as cpool, \
             tc.tile_pool(name="xin", bufs=4) as xpool, \
             tc.tile_pool(name="osb", bufs=4) as opool, \
             tc.tile_pool(name="ps", bufs=2, space="PSUM") as ppool:
            w_sb = cpool.tile([2 * K, 2 * C], mybir.dt.float16)
            nc.sync.dma_start(out=w_sb, in_=w2.ap())
            b_sb = cpool.tile([2 * C, 1], mybir.dt.float32)
            nc.sync.dma_start(out=b_sb, in_=br.ap())

            xt_ap = xt.ap()
            # out element [c, i*(2f) + j*f + s]  <->  AP dims [i, j, c, s]
            out_r = out.ap().rearrange("c (i j s) -> i j c s", i=n_outer, j=2)

            for it in range(n_outer):
                a0 = it * 2 * f
                x_tile = xpool.tile([2 * K, f], mybir.dt.float16)
                # Split each half-load into 48+1 rows: HWDGE fans a DMA's
                # descriptors over the largest engine count dividing the outer
                # dim (49 -> only 7 engines; 48 -> all 16).
                nc.sync.dma_start(out=x_tile[0:48, :], in_=xt_ap[0:48, a0:a0 + f])
                nc.sync.dma_start(out=x_tile[48:K, :], in_=xt_ap[48:K, a0:a0 + f])
                nc.sync.dma_start(out=x_tile[K:K + 48, :], in_=xt_ap[0:48, a0 + f:a0 + 2 * f])
                nc.sync.dma_start(out=x_tile[K + 48:2 * K, :], in_=xt_ap[48:K, a0 + f:a0 + 2 * f])
                for ob in range(f // osb):
                    o_tile = opool.tile([2 * C, osb], mybir.dt.float16)
                    for g in range(osb // group):
                        ps = ppool.tile([2 * C, group], mybir.dt.float32)
                        for n in range(group // nmm):
                            col = ob * osb + g * group + n * nmm
                            nc.tensor.matmul(
                                ps[:, n * nmm:(n + 1) * nmm],
                                w_sb,
                                x_tile[:, col:col + nmm],
                                start=True,
                                stop=True,
                            )
                        nc.scalar.activation(
                            o_tile[:, g * group:(g + 1) * group],
                            ps,
                            mybir.ActivationFunctionType.Identity,
                            bias=b_sb,
                            scale=1.0,
                        )
                    # One DMA per j-half: keeps the DRAM-side outer dim at 32
                    # (c), so HWDGE spreads descriptors across all engines
                    # instead of piling everything on two. Issued on the
                    # scalar-engine HWDGE ring so stores waiting on ACT can't
                    # head-of-line-block the input loads on the sync ring.
                    for j in range(2):
                        nc.scalar.dma_start(
                            out=out_r[it, j, :, ob * osb:(ob + 1) * osb],
                            in_=o_tile[j * C:(j + 1) * C, :],
                        )
    nc.compile()
    return nc


def _get_program():
    key = (W_CORE, F, OSB, GROUP, NMM)
    if key not in _PROGRAM_CACHE:
        _PROGRAM_CACHE[key] = build_program()
    return _PROGRAM_CACHE[key]


def prepare_inputs(enc_x, weight, bias):
    """Host-side prep: per-core transposed fp16 shards + block-diag weights."""
    enc_x = np.asarray(enc_x, dtype=np.float32)
    weight = np.asarray(weight, dtype=np.float32)
    bias = np.asarray(bias, dtype=np.float32)

    wflat = weight.reshape(C, K)
    w2 = np.zeros((2 * K, 2 * C), dtype=np.float16)
    w2[0:K, 0:C] = wflat.T.astype(np.float16)
    w2[K:2 * K, C:2 * C] = wflat.T.astype(np.float16)
    br = np.tile(bias, 2)[:, None].astype(np.float32)

    x16 = enc_x.astype(np.float16)
    shards = [
        np.ascontiguousarray(x16[i * W_CORE:(i + 1) * W_CORE].T)
        for i in range(N_CORES)
    ]
    return shards, w2, br


def kernel(enc_x, weight, bias, windows_nb=None):
    global LAST_RESULT
    from concourse import bass_utils

    shards, w2, br = prepare_inputs(enc_x, weight, bias)
    nc = _get_program()
    in_maps = [{"xt": shards[i], "w2": w2, "br": br} for i in range(N_CORES)]
    trace = bool(int(os.environ.get("BASS_KERNEL_TRACE", "0")))
    tmpdir = os.environ.get("BASS_KERNEL_TMPDIR") or None
    res = bass_utils.run_bass_kernel_spmd(
        nc, in_maps, core_ids=list(range(N_CORES)), trace=trace, tmpdir=tmpdir
    )
    LAST_RESULT = res
    outs = [res.results[i]["out"] for i in range(N_CORES)]
    return np.concatenate(outs, axis=1).astype(np.float32).reshape(-1)
